# revision 7
# baseline (speedup 1.0000x reference)
"""Corrected correlation loss on 8 Trainium2 NeuronCores.

loss = mean_{i<j} (corr(X) - corr(Y))[i,j]^2  for X, Y: [8192, 1024] f32,
where corr(A) standardizes columns (mean, std ddof=1, eps=1e-5) and takes
(As^T As)/n.

Strategy (data-parallel over the batch dim, 1024 rows/core):
  - per core: cast shard to bf16; column sums s and sums-of-squares q via
    ones-row matmuls accumulated in PSUM
  - 16KB AllReduce of [s,q]; stats math runs on a [128, 8] re-layout so all
    128 DVE lanes work; inv = 1/(sd+eps) stays fp32 (bf16-rounding inv is
    biased for its peaked-around-1.0 distribution; the loss shifts by ~4x
    the mean bias, ~2.5e-3)
  - scale columns As = bf16(A_f32 * inv); mean removal is folded into the
    Gram as one rank-1 "augmented row" matmul per matrix: the X side adds
    the *Y* mean-row outer product and vice versa, so D = AugGramX -
    AugGramY equals GsX - GsY exactly; rows carry sqrt(n/8) so the 8
    SPMD cores together contribute exactly 1x
  - Gram of only the 3 upper 512x512 superblocks (symmetry); diagonal
    superblocks pre-scaled by sqrt(1/2) so a plain full-block square-sum
    equals the strict-upper-triangle sum (D's diagonal entries are ~1e-3
    vs off-diag rms ~127; their half-weighted squares are negligible)
  - ReduceScatter of packed [1536, 512] f32 D-partials; each core squares
    and reduces its 192-row shard to a scalar partial
  - host sums the 8 partials and normalizes by n^2 * count
"""

import numpy as np

N_TOTAL = 8192
F = 1024
NCORES = 8
B = N_TOTAL // NCORES          # 1024 rows per core
KCH = B // 128                 # 8 chunks of 128 rows
EPS = 1e-5
COUNT = F * (F - 1) // 2
SUPERBLOCKS = [(0, 0, True), (0, 512, False), (512, 512, True)]
SQ_HALF = float(np.sqrt(0.5))

_compiled = {}


def build(no_collectives=False, num_devices=NCORES, nreps=1):
    # no_collectives=True swaps the collectives for local DMA copies
    # (wrong results, same engine workload) so single-core TimelineSim
    # can run. nreps>1 emits the body N times reusing the same tiles
    # (WAR deps serialize reps) for slope-based HW timing.
    import concourse.bacc as bacc
    import concourse.tile as tile
    import concourse.mybir as mybir

    f32 = mybir.dt.float32
    bf16 = mybir.dt.bfloat16
    mult = mybir.AluOpType.mult
    add = mybir.AluOpType.add
    subtract = mybir.AluOpType.subtract

    nc = bacc.Bacc("TRN2", target_bir_lowering=False, debug=False,
                   num_devices=num_devices)

    xs_in = nc.dram_tensor("xs", [B, F], f32, kind="ExternalInput").ap()
    ys_in = nc.dram_tensor("ys", [B, F], f32, kind="ExternalInput").ap()
    out_p = nc.dram_tensor("partial", [1, 1], f32, kind="ExternalOutput").ap()

    rg = [list(range(NCORES))]
    n = float(N_TOTAL)

    with tile.TileContext(nc) as tc:
        with tc.tile_pool(name="persist", bufs=1) as persist, \
             tc.tile_pool(name="dram", bufs=1, space="DRAM") as dram:

            ones_bf = persist.tile([128, 1], bf16)
            nc.vector.memset(ones_bf, 1.0)
            ones_f = persist.tile([128, 1], f32)
            nc.vector.memset(ones_f, 1.0)

            # persistent tiles, shared across reps
            raw = {(mat, k): persist.tile([128, F], f32,
                                          name=f"raw_{mat}_{k}")
                   for mat in range(2) for k in range(KCH)}
            asb = {(mat, k): persist.tile([128, F], bf16,
                                          name=f"asb_{mat}_{k}")
                   for mat in range(2) for k in range(KCH)}
            invb = {mat: persist.tile([128, F], f32, name=f"invb_{mat}")
                    for mat in range(2)}
            maug = {mat: persist.tile([1, F], bf16, name=f"maug_{mat}")
                    for mat in range(2)}
            stat_rows = {(mat, v): persist.tile([1, F], f32,
                                                name=f"stat_row_{mat}_{v}")
                         for mat in range(2) for v in range(2)}

            stats_in = dram.tile([4, F], f32)
            inv_row = {mat: dram.tile([1, F], f32, name=f"inv_row_{mat}")
                       for mat in range(2)}
            maug_row = {mat: dram.tile([1, F], bf16, name=f"maug_row_{mat}")
                        for mat in range(2)}
            d_dram = dram.tile([3 * 512, 512], f32)
            d_out = dram.tile([3 * 512 // NCORES, 512], f32)  # [192, 512]

            for rep in range(nreps):
                # Shared scratchpad tensors are write-once: fresh per rep.
                stats_out = dram.tile([4, F], f32, addr_space="Shared",
                                      name=f"stats_out_{rep}")
                # ---- load + cast + stats matmuls ------------------------
                with tc.tile_pool(name="ld", bufs=3) as ld, \
                     tc.tile_pool(name="statps", bufs=1,
                                  space="PSUM") as statps:
                    sps = {}
                    for mat in range(2):
                        for v in range(2):      # 0: sum, 1: sumsq
                            for h in range(2):  # column half
                                sps[(mat, v, h)] = statps.tile(
                                    [1, 512], f32, name=f"sps_{mat}_{v}_{h}")
                    for mat, src in ((0, xs_in), (1, ys_in)):
                        for k in range(KCH):
                            rk = raw[(mat, k)]
                            nc.sync.dma_start(
                                rk, src[k * 128:(k + 1) * 128, :])
                            ab = ld.tile([128, F], bf16, tag="ab")
                            nc.scalar.copy(ab, rk)        # f32 -> bf16
                            sq = ld.tile([128, F], bf16, tag="sq")
                            nc.scalar.square(sq, ab)
                            for h in range(2):
                                nc.tensor.matmul(
                                    sps[(mat, 0, h)], lhsT=ones_bf,
                                    rhs=ab[:, h * 512:(h + 1) * 512],
                                    start=(k == 0), stop=(k == KCH - 1))
                                nc.tensor.matmul(
                                    sps[(mat, 1, h)], lhsT=ones_bf,
                                    rhs=sq[:, h * 512:(h + 1) * 512],
                                    start=(k == 0), stop=(k == KCH - 1))
                    # stats psum -> 4 SBUF rows (partition 0)
                    for mat in range(2):
                        for v in range(2):
                            for h in range(2):
                                nc.scalar.copy(
                                    stat_rows[(mat, v)][
                                        :, h * 512:(h + 1) * 512],
                                    sps[(mat, v, h)])

                # ---- AllReduce the stats --------------------------------
                for mat in range(2):
                    for v in range(2):
                        nc.sync.dma_start(
                            stats_in[2 * mat + v:2 * mat + v + 1, :],
                            stat_rows[(mat, v)])
                if no_collectives:
                    nc.sync.dma_start(stats_out, stats_in)
                else:
                    nc.gpsimd.collective_compute(
                        "AllReduce", add, replica_groups=rg,
                        ins=[stats_in.opt()], outs=[stats_out.opt()])

                # ---- stats math on [128, 8] layout ----------------------
                # tile[p, c] = vec[128*c + p]
                with tc.tile_pool(name="stats", bufs=1) as stp:
                    for mat in range(2):
                        s_t = stp.tile([128, KCH], f32, name=f"s_t_{mat}")
                        q_t = stp.tile([128, KCH], f32, name=f"q_t_{mat}")
                        nc.sync.dma_start(
                            s_t,
                            stats_out[2 * mat:2 * mat + 1, :].rearrange(
                                "o (c p) -> p (o c)", p=128))
                        nc.sync.dma_start(
                            q_t,
                            stats_out[2 * mat + 1:2 * mat + 2, :].rearrange(
                                "o (c p) -> p (o c)", p=128))
                        mu_t = stp.tile([128, KCH], f32, name=f"mu_t_{mat}")
                        t0 = stp.tile([128, KCH], f32, name=f"t0_{mat}")
                        t1 = stp.tile([128, KCH], f32, name=f"t1_{mat}")
                        inv_t = stp.tile([128, KCH], f32,
                                         name=f"inv_t_{mat}")
                        ma_t = stp.tile([128, KCH], bf16, name=f"ma_t_{mat}")
                        nc.vector.tensor_scalar_mul(mu_t, s_t, 1.0 / n)
                        nc.vector.tensor_mul(t0, mu_t, mu_t)
                        nc.vector.tensor_scalar_mul(t1, q_t, 1.0 / (n - 1.0))
                        nc.vector.scalar_tensor_tensor(
                            t0, in0=t0, scalar=-n / (n - 1.0), in1=t1,
                            op0=mult, op1=add)             # var
                        nc.scalar.sqrt(t0, t0)             # sd
                        nc.vector.tensor_scalar_add(t0, t0, EPS)
                        nc.vector.reciprocal(inv_t, t0)    # fp32 1/(sd+eps)
                        nc.vector.tensor_mul(t1, mu_t, inv_t)
                        nc.vector.tensor_scalar_mul(
                            ma_t, t1, float(np.sqrt(n / NCORES)))
                        nc.sync.dma_start(
                            inv_row[mat][:, :].rearrange(
                                "o (c p) -> p (o c)", p=128), inv_t)
                        nc.sync.dma_start(
                            maug_row[mat][:, :].rearrange(
                                "o (c p) -> p (o c)", p=128), ma_t)

                # broadcast inv to all partitions; aug rows to partition 0
                for mat in range(2):
                    nc.sync.dma_start(
                        invb[mat],
                        inv_row[mat][0:1, :].to_broadcast([128, F]))
                    nc.sync.dma_start(maug[mat], maug_row[mat][:, :])

                # ---- scale pass: asb = bf16(raw * invb) -----------------
                for mat in range(2):
                    for k in range(KCH):
                        nc.vector.tensor_mul(asb[(mat, k)], raw[(mat, k)],
                                             invb[mat])

                # ---- Gram superblocks + subtract + stage to DRAM --------
                with tc.tile_pool(name="gps", bufs=2, space="PSUM") as gps, \
                     tc.tile_pool(name="dst", bufs=3) as dst:
                    for sbi, (rb, cb, diag) in enumerate(SUPERBLOCKS):
                        w = SQ_HALF if diag else 1.0
                        for m in range(4):
                            r0 = rb + m * 128
                            psx = gps.tile([128, 512], f32, tag="psx")
                            psy = gps.tile([128, 512], f32, tag="psy")
                            for k in range(KCH):
                                nc.tensor.matmul(
                                    psx, lhsT=asb[(0, k)][:, r0:r0 + 128],
                                    rhs=asb[(0, k)][:, cb:cb + 512],
                                    start=(k == 0), stop=False)
                            # aug row: X side adds the Y mean outer product
                            nc.tensor.matmul(
                                psx, lhsT=maug[1][:, r0:r0 + 128],
                                rhs=maug[1][:, cb:cb + 512],
                                start=False, stop=True)
                            for k in range(KCH):
                                nc.tensor.matmul(
                                    psy, lhsT=asb[(1, k)][:, r0:r0 + 128],
                                    rhs=asb[(1, k)][:, cb:cb + 512],
                                    start=(k == 0), stop=False)
                            nc.tensor.matmul(
                                psy, lhsT=maug[0][:, r0:r0 + 128],
                                rhs=maug[0][:, cb:cb + 512],
                                start=False, stop=True)
                            # d = w*psx - w*psy (ACT scales psy on copy)
                            yt = dst.tile([128, 512], f32, tag="yt")
                            nc.scalar.mul(yt, psy, w)
                            dt_ = dst.tile([128, 512], f32, tag="dt")
                            nc.vector.scalar_tensor_tensor(
                                dt_, in0=psx, scalar=w, in1=yt,
                                op0=mult, op1=subtract)
                            nc.sync.dma_start(
                                d_dram[sbi * 512 + m * 128:
                                       sbi * 512 + (m + 1) * 128, :], dt_)

                # ---- ReduceScatter --------------------------------------
                if no_collectives:
                    nc.sync.dma_start(
                        d_out, d_dram[0:3 * 512 // NCORES, :])
                else:
                    nc.gpsimd.collective_compute(
                        "ReduceScatter", add, replica_groups=rg,
                        ins=[d_dram.opt()], outs=[d_out.opt()])

                # ---- square + reduce my shard ---------------------------
                with tc.tile_pool(name="post", bufs=1) as post, \
                     tc.tile_pool(name="postps", bufs=1,
                                  space="PSUM") as postps:
                    shard0 = post.tile([128, 512], f32)
                    shard1 = post.tile([64, 512], f32)
                    nc.sync.dma_start(shard0, d_out[0:128, :])
                    nc.sync.dma_start(shard1, d_out[128:192, :])
                    sqs0 = post.tile([128, 512], f32)
                    sqs1 = post.tile([64, 512], f32)
                    rs0 = post.tile([128, 1], f32)
                    rs1 = post.tile([64, 1], f32)
                    nc.scalar.activation(
                        sqs0, shard0, mybir.ActivationFunctionType.Square,
                        accum_out=rs0)
                    nc.scalar.activation(
                        sqs1, shard1, mybir.ActivationFunctionType.Square,
                        accum_out=rs1)
                    tot = postps.tile([1, 1], f32)
                    nc.tensor.matmul(tot, lhsT=ones_f, rhs=rs0,
                                     start=True, stop=False)
                    nc.tensor.matmul(tot, lhsT=ones_f[0:64, :], rhs=rs1,
                                     start=False, stop=True)
                    res_sb = post.tile([1, 1], f32)
                    nc.scalar.copy(res_sb, tot)
                    nc.sync.dma_start(out_p, res_sb)

    nc.compile()
    return nc


def kernel(X: np.ndarray, Y: np.ndarray) -> np.ndarray:
    import concourse.bass_utils as bass_utils

    X = np.ascontiguousarray(np.asarray(X, dtype=np.float32))
    Y = np.ascontiguousarray(np.asarray(Y, dtype=np.float32))
    assert X.shape == (N_TOTAL, F) and Y.shape == (N_TOTAL, F)

    if "nc" not in _compiled:
        _compiled["nc"] = build()
    nc = _compiled["nc"]

    in_maps = []
    for c in range(NCORES):
        in_maps.append({
            "xs": X[c * B:(c + 1) * B],
            "ys": Y[c * B:(c + 1) * B],
        })
    res = bass_utils.run_bass_kernel_spmd(
        nc, in_maps, core_ids=list(range(NCORES)))
    total = 0.0
    for c in range(NCORES):
        total += float(res.results[c]["partial"][0, 0])
    loss = total / (float(N_TOTAL) ** 2 * COUNT)
    return np.float32(loss)
